# revision 19
# baseline (speedup 1.0000x reference)
"""Distributed Trainium2 Bass kernel for causal GQA attention block.

Problem (hardcoded): x [4, 2048, 1024] f32; wq [1024, 1024]; wk/wv [1024, 256];
wo [1024, 1024]. 16 q-heads, 4 kv-heads, head_dim 64, rms-norm on q/k (no
weight), rope (base 10000), q gain 1.5, causal SDPA, out-proj.

Sharding over 8 cores: core i -> batch b = i//2, head-half p = i%2
(q-heads 8p..8p+7, kv-heads 2p, 2p+1 -- KV groups intact).

v2 design vs baseline:
- x is pre-transposed on the host (contiguous feature-major load, no DMA
  transpose); weights pre-arranged so all loads are contiguous.
- Emission is pipelined: K/V + k-rope first, then per m-group Q proj ->
  rope/rms -> attention(j=0, m-1), so the Scalar engine's softmax EXP starts
  ~30us in instead of after all projections.
- rms stats are packed [8, 512] (token-blocks in partitions) so Ln/Exp on
  the Scalar engine cost ~0.5us per chunk instead of ~3.4us; Act tables
  are pinned to the one set holding both Ln and Exp (no reload thrash).
- Normalization: stage the PSUM rowsum row to SBUF partition 0 (the
  custom-DVE reciprocal only honors base partition 0), reciprocal, bf16
  cast, gpsimd partition-broadcast, then two multiplies reading the O
  rows straight from PSUM (no [65,1024] eviction copy).
- Out-projection is reformulated as pairwise ReduceScatter: each core
  computes partial y for BOTH column-halves from its own heads' O^T
  (weights host-ordered so the program is SPMD-uniform), the pair
  ReduceScatter-adds, and each core's shard lands in its out slice.  No
  AllGather of O^T, no og reload, and the collective is off the critical
  path (out-proj never waits on it).
"""
import sys

sys.path.insert(0, "/opt/trn_rl_repo")

import numpy as np
import ml_dtypes

import concourse.bacc as bacc
import concourse.mybir as mybir
import concourse.tile as tile
from concourse.bass_utils import run_bass_kernel_spmd

F32 = mybir.dt.float32
BF16 = mybir.dt.bfloat16
FP8 = mybir.dt.float8e4
AF = mybir.ActivationFunctionType
DR = mybir.MatmulPerfMode.DoubleRow

N = 2048          # tokens
C = 1024          # model dim
DQ = 512          # local q out-features (8 heads x 64)
DKV = 128         # local kv out-features (2 kv heads x 64)
D = 64            # head dim
NCC = C // 128    # 8 contraction chunks
NQT = 4           # q tiles of 512
NTC = N // 128    # 16 token chunks
QK_GAIN = 1.5
EXP_SCALE = QK_GAIN / np.sqrt(D).item()  # folded gain * 1/sqrt(D) = 0.1875
EPS = float(np.finfo(np.float32).eps)
WSCALE = 64.0     # fp8 weight pre-scale; evictions multiply by 1/64
ROPE_BASE = 10000.0
PAIRS = [[0, 1], [2, 3], [4, 5], [6, 7]]


def _host_tables():
    inv_freq = (1.0 / (ROPE_BASE ** (np.arange(0, D, 2, dtype=np.float64) / D)))
    t = np.arange(N, dtype=np.float64)
    ang = np.outer(inv_freq, t)  # [32, N]
    cos32 = np.cos(ang)
    sin32 = np.sin(ang)
    cosT = np.tile(cos32, (4, 1)).astype(np.float32)  # [128, N]
    sinTs = np.concatenate([-sin32, sin32, -sin32, sin32], axis=0).astype(np.float32)
    q = np.arange(128)
    trimask = (q[None, :] >= q[:, None]).astype(np.float32)  # keep q >= k
    ident = np.eye(128, dtype=np.float32)
    # lnmm lhsT: per qt, col (2qt+h) sums head-h partitions; other cols zero
    ones4 = np.zeros((128, NQT, 8), np.float32)
    for qt in range(NQT):
        ones4[0:64, qt, 2 * qt] = 1.0
        ones4[64:128, qt, 2 * qt + 1] = 1.0
    # expander lhsT: per qt, out rows p<64 <- rr2 row 2qt, p>=64 <- row 2qt+1
    expL = np.zeros((8, NQT, 128), np.float32)
    for qt in range(NQT):
        expL[2 * qt, qt, 0:64] = 1.0
        expL[2 * qt + 1, qt, 64:128] = 1.0
    bf = ml_dtypes.bfloat16
    return {
        "cosT": cosT.astype(bf),
        "sinTs": sinTs.astype(bf),
        "trimask": trimask.astype(bf),
        "ident": ident.astype(bf),
        "ones4": ones4.astype(bf),
        "expL": expL.astype(bf),
    }


DEBUG = False


def _pin_act_tables(nc):
    """All Act funcs used here (Ln, Exp, Copy, Identity) live together in the
    'natural_log_exp_and_others' set, but the auto-placer matches Exp to
    'exp_and_others' and Ln to 'natural_log', reloading tables (~1.3us) around
    every Ln.  Blank the redundant sets' contents (positions/ids unchanged, so
    the runtime mapping into act_info.json stays valid) so every activation
    resolves to the one shared set and the table loads once."""
    from concourse.hw_specs import get_activation_tables
    tabs = get_activation_tables(nc.m.arch)
    keep = "natural_log_exp_and_others"
    if keep in tabs:
        for name_, funcs in tabs.items():
            if name_ != keep:
                funcs.clear()


def build():
    nc = bacc.Bacc(None, target_bir_lowering=False, num_devices=8)
    _pin_act_tables(nc)

    # host supplies x pre-transposed [C, N] bf16; weights pre-arranged
    x_ext = nc.declare_dram_parameter("x", [C, N], BF16, isOutput=False)
    wq_ext = nc.declare_dram_parameter("wq", [128, NCC, DQ], BF16, isOutput=False)
    wk_ext = nc.declare_dram_parameter("wk", [128, NCC, DKV], BF16, isOutput=False)
    wv_ext = nc.declare_dram_parameter("wv", [128, NCC, DKV], BF16, isOutput=False)
    wo_ext = nc.declare_dram_parameter("wo", [128, 2, 4, DQ], BF16, isOutput=False)
    out_ext = nc.declare_dram_parameter("out", [N, DQ], BF16, isOutput=True)
    if DEBUG:
        dbg_q = nc.declare_dram_parameter("dbg_q", [128, NQT, N], BF16, isOutput=True)
        dbg_kA = nc.declare_dram_parameter("dbg_kA", [128, N], BF16, isOutput=True)
        dbg_kB = nc.declare_dram_parameter("dbg_kB", [128, N], BF16, isOutput=True)
        dbg_v = nc.declare_dram_parameter("dbg_v", [128, NTC, 130], BF16, isOutput=True)
        dbg_o = nc.declare_dram_parameter("dbg_o", [128, 4, N], BF16, isOutput=True)
        dbg_r = nc.declare_dram_parameter("dbg_r", [16, 2, 512], F32, isOutput=True)

    tabs = _host_tables()
    cosT_d = nc.inline_tensor(tabs["cosT"], name="cosT_d")
    sinTs_d = nc.inline_tensor(tabs["sinTs"], name="sinTs_d")
    trimask_d = nc.inline_tensor(tabs["trimask"], name="trimask_d")
    ident_d = nc.inline_tensor(tabs["ident"], name="ident_d")
    ones4_d = nc.inline_tensor(tabs["ones4"], name="ones4_d")
    expL_d = nc.inline_tensor(tabs["expL"], name="expL_d")

    with tile.TileContext(nc) as tc:
        with (
            tc.tile_pool(name="dram", bufs=1, space="DRAM") as dram,
            tc.tile_pool(name="persist", bufs=1) as ps,
        ):
            # ---- persistent SBUF tensors ----
            xT = ps.tile([128, NCC, N], BF16, name="xT")
            wq_sb = ps.tile([128, NCC, DQ], BF16, name="wq_sb")
            wk_sb = ps.tile([128, NCC, DKV], BF16, name="wk_sb")
            wv_sb = ps.tile([128, NCC, DKV], BF16, name="wv_sb")
            wo_sb = ps.tile([128, 2, 4, DQ], BF16, name="wo_sb")
            cosT = ps.tile([128, N], BF16, name="cosT")
            sinTs = ps.tile([128, N], BF16, name="sinTs")
            trimask = ps.tile([128, 128], BF16, name="trimask")
            ident = ps.tile([128, 128], BF16, name="ident")
            ones4 = ps.tile([128, NQT, 8], BF16, name="ones4")
            expL = ps.tile([8, NQT, 128], BF16, name="expL")
            eps_sb = ps.tile([128, 1], F32, name="eps_sb")
            kT_raw = ps.tile([128, N], BF16, name="kT_raw")
            kTdA = ps.tile([128, N], BF16, name="kTdA")
            kTdB = ps.tile([128, N], BF16, name="kTdB")
            vT = ps.tile([128, N], BF16, name="vT")
            v_sb = ps.tile([128, NTC, 130], BF16, name="v_sb")   # [V_A|1|V_B|1]
            qT = ps.tile([128, NQT, N], BF16, name="qT")
            oT = ps.tile([128, 4, N], BF16, name="oT")           # normalized O^T

            # ---- stage inputs ----
            for cc in range(NCC):
                nc.sync.dma_start(
                    out=xT[:, cc, :], in_=x_ext[cc * 128:(cc + 1) * 128, :]
                )
            nc.gpsimd.dma_start(out=wk_sb[:], in_=wk_ext[:])
            nc.gpsimd.dma_start(out=wv_sb[:], in_=wv_ext[:])
            nc.gpsimd.dma_start(out=wq_sb[:], in_=wq_ext[:])
            nc.gpsimd.dma_start(out=cosT[:], in_=cosT_d[:])
            nc.gpsimd.dma_start(out=sinTs[:], in_=sinTs_d[:])
            nc.gpsimd.dma_start(out=trimask[:], in_=trimask_d[:])
            nc.gpsimd.dma_start(out=ident[:], in_=ident_d[:])
            nc.gpsimd.dma_start(out=ones4[:], in_=ones4_d[:])
            nc.gpsimd.dma_start(out=expL[:], in_=expL_d[:])
            nc.gpsimd.dma_start(out=wo_sb[:], in_=wo_ext[:])
            nc.gpsimd.memset(eps_sb[:], EPS)
            nc.gpsimd.memset(v_sb[:, :, 64:65], 1.0)
            nc.gpsimd.memset(v_sb[:, :, 129:130], 1.0)

            rs_ins = [dram.tile([2, 128, DQ], BF16, name=f"rs_in{t}") for t in range(NTC)]
            rs_outs = [
                dram.tile([128, DQ], BF16, name=f"rs_out{t}") for t in range(NTC)
            ]

            with (
                tc.tile_pool(name="u_psum", bufs=1, space="PSUM") as up,
                tc.tile_pool(name="u_sbuf", bufs=3) as bs,
            ):
                # ---------- helpers ----------
                def emit_proj(w_sb, nf_off, nf, dst, tag_base):
                    """project xT against w_sb[:, :, nf_off:nf_off+nf] and
                    evict to dst [128, N] bf16."""
                    for qp in range(2):
                        pp = up.tile(
                            [128, 2, 512], F32,
                            tag=("mm" if qp % 2 == 0 else "o"), bufs=2,
                            name=f"pp{tag_base}",
                        )
                        for h in range(2):
                            qt = 2 * qp + h
                            for cc in range(NCC):
                                nc.tensor.matmul(
                                    pp[:, h, :],
                                    w_sb[:, cc, nf_off:nf_off + nf],
                                    xT[:, cc, qt * 512:(qt + 1) * 512],
                                    start=(cc == 0), stop=(cc == NCC - 1),
                                )
                        nc.vector.tensor_copy(
                            dst[:, qp * 1024:(qp + 1) * 1024],
                            pp.rearrange("p a b -> p (a b)"),
                        )

                def emit_rms_rope(src, dst, ci):
                    """rms-norm (no weight) + rope, feature-major, in place ok."""
                    sq = bs.tile([128, N], BF16, tag="sq", bufs=2, name=f"sq{ci}")
                    nc.vector.tensor_mul(sq[:], src, src)
                    msp = up.tile([8, 512], F32, tag="mm", bufs=2, name=f"msp{ci}")
                    for qt in range(NQT):
                        nc.tensor.matmul(
                            msp[:], ones4[:, qt, :], sq[:, qt * 512:(qt + 1) * 512],
                            start=(qt == 0), stop=(qt == NQT - 1),
                        )
                    lnv = bs.tile([8, 512], F32, tag="lnv", bufs=2, name=f"lnv{ci}")
                    nc.scalar.activation(
                        lnv[:], msp[:], AF.Ln, bias=eps_sb[0:8, :], scale=1.0 / D
                    )
                    rr2 = bs.tile([8, 512], BF16, tag="rr2", bufs=2, name=f"rr2{ci}")
                    nc.scalar.activation(rr2[:], lnv[:], AF.Exp, scale=-0.5)
                    t1 = bs.tile([128, N], BF16, tag="t1", bufs=2, name=f"t1{ci}")
                    nc.vector.tensor_mul(t1[:], src, cosT[:])
                    t2 = bs.tile([128, N], BF16, tag="t2", bufs=2, name=f"t2{ci}")
                    nc.vector.tensor_copy(t2[0:32, :], src[32:64, :])
                    nc.vector.tensor_copy(t2[32:64, :], src[0:32, :])
                    nc.vector.tensor_copy(t2[64:96, :], src[96:128, :])
                    nc.vector.tensor_copy(t2[96:128, :], src[64:96, :])
                    nc.vector.tensor_mul(t2[:], t2[:], sinTs[:])
                    nc.vector.tensor_add(t1[:], t1[:], t2[:])
                    for qt in range(NQT):
                        rbp = up.tile(
                            [128, 512], F32, tag="o", bufs=2, name=f"rbp{ci}"
                        )
                        nc.tensor.matmul(
                            rbp[:], expL[:, qt, :], rr2[:],
                            start=True, stop=True,
                        )
                        nc.vector.tensor_mul(
                            dst[:, qt * 512:(qt + 1) * 512],
                            t1[:, qt * 512:(qt + 1) * 512], rbp[:],
                        )

                def emit_attention(j, m):
                    kT = kTdA if m < 2 else kTdB
                    vslot = 0 if m < 2 else 65
                    oab = up.tile([65, 2, 512], F32, tag="o", bufs=2, name=f"oab{j}_{m}")
                    nkc = 4 * (j + 1)

                    def emit_scores(kc):
                        i = kc - 4 * j
                        off = max(0, 128 * i)
                        w = 512 - off
                        q0 = 512 * j + off
                        sAB = up.tile([128, 2, 512], F32, tag="mm", bufs=2, name=f"sAB{kc}")
                        nc.tensor.matmul(
                            sAB[:, 0, 0:w], kT[0:64, kc * 128:(kc + 1) * 128],
                            qT[0:64, m, q0:q0 + w], start=True, stop=True,
                            tile_position=(0, 0),
                        )
                        nc.tensor.matmul(
                            sAB[:, 1, 0:w], kT[64:128, kc * 128:(kc + 1) * 128],
                            qT[64:128, m, q0:q0 + w], start=True, stop=True,
                            tile_position=(64, 0),
                        )
                        pAB = bs.tile([128, 2, 512], BF16, tag="pAB", bufs=5, name=f"pAB{kc}")
                        nc.scalar.activation(
                            pAB[:, :, 0:w], sAB[:, :, 0:w], AF.Exp, scale=EXP_SCALE
                        )
                        if i >= 0:
                            nc.vector.tensor_mul(
                                pAB[:, :, 0:128], pAB[:, :, 0:128],
                                trimask.rearrange("p (a b) -> p a b", a=1).broadcast_to([128, 2, 128]),
                            )
                        return pAB

                    def emit_pv(kc, pAB):
                        i = kc - 4 * j
                        off = max(0, 128 * i)
                        w = 512 - off
                        nc.tensor.matmul(
                            oab[:, 0, off:512], v_sb[:, kc, vslot:vslot + 65],
                            pAB[:, 0, 0:w], start=(kc == 0), stop=(kc == nkc - 1),
                            skip_group_check=True,
                        )
                        nc.tensor.matmul(
                            oab[:, 1, off:512], v_sb[:, kc, vslot:vslot + 65],
                            pAB[:, 1, 0:w], start=(kc == 0), stop=(kc == nkc - 1),
                            skip_group_check=True,
                        )

                    staged = []
                    for kc in range(nkc):
                        staged.append((kc, emit_scores(kc)))
                        if len(staged) == 2:
                            for kcx, px in staged:
                                emit_pv(kcx, px)
                            staged = []
                    for kcx, px in staged:
                        emit_pv(kcx, px)
                    # normalize: r = 1/rowsum broadcast over 64 partitions
                    ssum = bs.tile([1, 2, 512], F32, tag="ssum", bufs=2, name=f"ssum{j}_{m}")
                    nc.vector.tensor_copy(ssum[:], oab[64:65, :, :])
                    rrf = bs.tile([1, 2, 512], F32, tag="rrf", bufs=2, name=f"rrf{j}_{m}")
                    nc.vector.reciprocal_approx_fast(rrf[:], ssum[:])
                    rrb = bs.tile([1, 2, 512], BF16, tag="rrb", bufs=2, name=f"rrb{j}_{m}")
                    nc.vector.tensor_copy(rrb[:], rrf[:])
                    if DEBUG:
                        nc.sync.dma_start(out=dbg_r[4 * j + m], in_=rrf[0, :, :])
                    rbs = bs.tile([64, 1024], BF16, tag="rbs", bufs=2, name=f"rbs{j}_{m}")
                    nc.gpsimd.partition_broadcast(
                        rbs[:], rrb.rearrange("p a b -> p (a b)"), channels=64
                    )
                    nc.vector.tensor_mul(
                        oT[0:64, m, 512 * j:512 * (j + 1)], oab[0:64, 0, :], rbs[:, 0:512]
                    )
                    nc.vector.tensor_mul(
                        oT[64:128, m, 512 * j:512 * (j + 1)], oab[0:64, 1, :], rbs[:, 512:1024]
                    )

                def emit_outproj(j, tt):
                    """partial y for both column-halves from own heads, then
                    pairwise ReduceScatter-add; shard s goes to rank s."""
                    tcix = j * 4 + tt
                    po = up.tile([128, 2, 512], F32, tag="o", bufs=2, name=f"po{tcix}")
                    for s in range(2):
                        for rc in range(4):
                            nc.tensor.matmul(
                                po[:, s, :],
                                oT[:, rc, tcix * 128:(tcix + 1) * 128],
                                wo_sb[:, s, rc, :],
                                start=(rc == 0), stop=(rc == 3),
                            )
                    ev = bs.tile([128, 2, 512], BF16, tag="ev", bufs=2, name=f"ev{tcix}")
                    nc.vector.tensor_copy(ev[:, 0, :], po[:, 0, :])
                    nc.scalar.copy(ev[:, 1, :], po[:, 1, :])
                    nc.sync.dma_start(
                        out=rs_ins[tcix].rearrange("s p f -> p s f"), in_=ev[:]
                    )
                    nc.gpsimd.collective_compute(
                        "ReduceScatter",
                        mybir.AluOpType.add,
                        replica_groups=PAIRS,
                        ins=[rs_ins[tcix].opt()],
                        outs=[rs_outs[tcix].opt()],
                    )
                    nc.sync.dma_start(
                        out=out_ext[tcix * 128:(tcix + 1) * 128, :],
                        in_=rs_outs[tcix][:],
                    )

                # ---------- emission ----------
                # K/V projections + k rope/rms + V transpose
                emit_proj(wk_sb, 0, DKV, kT_raw, "k")
                emit_proj(wv_sb, 0, DKV, vT, "v")
                for tcix in range(NTC):
                    pv = up.tile(
                        [128, 128], BF16,
                        tag=("mm" if tcix % 2 == 0 else "o"), bufs=2, name="pv",
                    )
                    nc.tensor.transpose(pv[:], vT[:, tcix * 128:(tcix + 1) * 128], ident[:])
                    nc.vector.tensor_copy(v_sb[:, tcix, 0:64], pv[:, 0:64])
                    nc.vector.tensor_copy(v_sb[:, tcix, 65:129], pv[:, 64:128])
                emit_rms_rope(kT_raw[:], kTdA[:], 4)
                # duplicate kv halves: kTdA holds [A; B] -> kTdA=[A;A], kTdB=[B;B]
                nc.vector.tensor_copy(kTdB[0:64, :], kTdA[64:128, :])
                nc.vector.tensor_copy(kTdB[64:128, :], kTdA[64:128, :])
                nc.vector.tensor_copy(kTdA[64:128, :], kTdA[0:64, :])

                # Q per m-group, interleaved with first attention tile
                for m in range(4):
                    emit_proj(wq_sb, m * 128, 128, qT[:, m, :], f"q{m}")
                    emit_rms_rope(qT[:, m, :], qT[:, m, :], m)
                    if m > 0:
                        emit_attention(0, m - 1)
                emit_attention(0, 3)

                for j in range(1, NQT):
                    for m in range(4):
                        emit_attention(j, m)
                        emit_outproj(j - 1, m)
                for tt in range(4):
                    emit_outproj(NQT - 1, tt)
                if DEBUG:
                    nc.sync.dma_start(out=dbg_q[:], in_=qT[:])
                    nc.sync.dma_start(out=dbg_kA[:], in_=kTdA[:])
                    nc.sync.dma_start(out=dbg_kB[:], in_=kTdB[:])
                    nc.sync.dma_start(out=dbg_v[:], in_=v_sb[:])
                    nc.sync.dma_start(out=dbg_o[:], in_=oT[:])

    nc.finalize()
    return nc


_NC_CACHE = None


def _get_nc():
    global _NC_CACHE
    if _NC_CACHE is None:
        _NC_CACHE = build()
    return _NC_CACHE


def _make_in_maps(inputs):
    x = np.asarray(inputs["x"], dtype=np.float32)
    wq = np.asarray(inputs["wq"], dtype=np.float32)
    wk = np.asarray(inputs["wk"], dtype=np.float32)
    wv = np.asarray(inputs["wv"], dtype=np.float32)
    wo = np.asarray(inputs["wo"], dtype=np.float32)
    bf = ml_dtypes.bfloat16

    def warr(w, p, nf):
        """[C, nf] local slice -> [128, NCC, nf] bf16"""
        wl = w[:, p * nf:(p + 1) * nf]
        return np.ascontiguousarray(
            wl.reshape(NCC, 128, nf).transpose(1, 0, 2).astype(bf)
        )

    in_maps = []
    for i in range(8):
        b, p = i // 2, i % 2
        wo_l = np.empty((128, 2, 4, DQ), dtype=bf)
        for s in range(2):
            for rc in range(4):
                wo_l[:, s, rc, :] = wo[
                    p * DQ + rc * 128:p * DQ + (rc + 1) * 128,
                    s * DQ:(s + 1) * DQ,
                ].astype(bf)
        in_maps.append({
            "x": np.ascontiguousarray(x[b].T.astype(bf)),
            "wq": warr(wq, p, DQ),
            "wk": warr(wk, p, DKV),
            "wv": warr(wv, p, DKV),
            "wo": np.ascontiguousarray(wo_l),
        })
    return in_maps


def kernel(x, wq, wk, wv, wo):
    x = np.asarray(x, dtype=np.float32)
    B = x.shape[0]
    nc = _get_nc()
    in_maps = _make_in_maps({"x": x, "wq": wq, "wk": wk, "wv": wv, "wo": wo})
    res = run_bass_kernel_spmd(nc, in_maps, core_ids=list(range(8)))
    out = np.empty((B, N, C), dtype=np.float32)
    for b in range(B):
        out[b, :, 0:DQ] = res.results[2 * b]["out"].astype(np.float32)
        out[b, :, DQ:C] = res.results[2 * b + 1]["out"].astype(np.float32)
    return out


if __name__ == "__main__":
    rng = np.random.default_rng(0)
    ins = {
        "x": rng.standard_normal((4, N, C), dtype=np.float32),
        "wq": (rng.standard_normal((C, C), dtype=np.float32) * 0.02),
        "wk": (rng.standard_normal((C, 256), dtype=np.float32) * 0.02),
        "wv": (rng.standard_normal((C, 256), dtype=np.float32) * 0.02),
        "wo": (rng.standard_normal((C, C), dtype=np.float32) * 0.02),
    }
    y = kernel(**ins)
    print("out", y.shape, y.dtype, np.abs(y).mean())


# revision 25
# speedup vs baseline: 1.3246x; 1.3246x over previous
"""Distributed Trainium2 Bass kernel for causal GQA attention block.

Problem (hardcoded): x [4, 2048, 1024] f32; wq [1024, 1024]; wk/wv [1024, 256];
wo [1024, 1024]. 16 q-heads, 4 kv-heads, head_dim 64, rms-norm on q/k (no
weight), rope (base 10000), q gain 1.5, causal SDPA, out-proj.

Sharding over 8 cores: core i -> batch b = i//2, head-half p = i%2
(q-heads 8p..8p+7, kv-heads 2p, 2p+1 -- KV groups intact).

v2 design vs baseline:
- x is pre-transposed on the host (contiguous feature-major load, no DMA
  transpose); weights pre-arranged so all loads are contiguous.
- Emission is pipelined: K/V + k-rope first, then per m-group Q proj ->
  rope/rms -> attention(j=0, m-1), so the Scalar engine's softmax EXP starts
  ~30us in instead of after all projections.
- rms stats are packed [8, 512] (token-blocks in partitions) so Ln/Exp on
  the Scalar engine cost ~0.5us per chunk instead of ~3.4us; Act tables
  are pinned to the one set holding both Ln and Exp (no reload thrash).
- Normalization: stage the PSUM rowsum row to SBUF partition 0 (the
  custom-DVE reciprocal only honors base partition 0), reciprocal, bf16
  cast, gpsimd partition-broadcast, then two multiplies reading the O
  rows straight from PSUM (no [65,1024] eviction copy).
- Out-projection is reformulated as pairwise ReduceScatter: each core
  computes partial y for BOTH column-halves from its own heads' O^T
  (weights host-ordered so the program is SPMD-uniform), the pair
  ReduceScatter-adds, and each core's shard lands in its out slice.  No
  AllGather of O^T, no og reload, and the collective is off the critical
  path (out-proj never waits on it).
"""
import sys

sys.path.insert(0, "/opt/trn_rl_repo")

import numpy as np
import ml_dtypes

import concourse.bacc as bacc
import concourse.mybir as mybir
import concourse.tile as tile
from concourse.bass_utils import run_bass_kernel_spmd

F32 = mybir.dt.float32
BF16 = mybir.dt.bfloat16
FP8 = mybir.dt.float8e4
AF = mybir.ActivationFunctionType
DR = mybir.MatmulPerfMode.DoubleRow

N = 2048          # tokens
C = 1024          # model dim
DQ = 512          # local q out-features (8 heads x 64)
DKV = 128         # local kv out-features (2 kv heads x 64)
D = 64            # head dim
NCC = C // 128    # 8 contraction chunks
NQT = 4           # q tiles of 512
NTC = N // 128    # 16 token chunks
QK_GAIN = 1.5
EXP_SCALE = QK_GAIN / np.sqrt(D).item()  # folded gain * 1/sqrt(D) = 0.1875
EPS = float(np.finfo(np.float32).eps)
WSCALE = 64.0     # fp8 weight pre-scale; evictions multiply by 1/64
ROPE_BASE = 10000.0
PAIRS = [[0, 1], [2, 3], [4, 5], [6, 7]]


def _host_tables():
    inv_freq = (1.0 / (ROPE_BASE ** (np.arange(0, D, 2, dtype=np.float64) / D)))
    t = np.arange(N, dtype=np.float64)
    ang = np.outer(inv_freq, t)  # [32, N]
    cos32 = np.cos(ang)
    sin32 = np.sin(ang)
    cosT = np.tile(cos32, (4, 1)).astype(np.float32)  # [128, N]
    sinTs = np.concatenate([-sin32, sin32, -sin32, sin32], axis=0).astype(np.float32)
    q = np.arange(128)
    trimask = (q[None, :] >= q[:, None]).astype(np.float32)  # keep q >= k
    ident = np.eye(128, dtype=np.float32)
    # lnmm lhsT: per qt, col (2qt+h) sums head-h partitions; other cols zero
    ones4 = np.zeros((128, NQT, 8), np.float32)
    for qt in range(NQT):
        ones4[0:64, qt, 2 * qt] = 1.0
        ones4[64:128, qt, 2 * qt + 1] = 1.0
    # expander lhsT: per qt, out rows p<64 <- rr2 row 2qt, p>=64 <- row 2qt+1
    expL = np.zeros((8, NQT, 128), np.float32)
    for qt in range(NQT):
        expL[2 * qt, qt, 0:64] = 1.0
        expL[2 * qt + 1, qt, 64:128] = 1.0
    bf = ml_dtypes.bfloat16
    return {
        "cosT": cosT.astype(bf),
        "sinTs": sinTs.astype(bf),
        "trimask": trimask.astype(bf),
        "ident": ident.astype(bf),
        "ones4": ones4.astype(bf),
        "expL": expL.astype(bf),
    }


DEBUG = False


def _pin_act_tables(nc):
    """All Act funcs used here (Ln, Exp, Copy, Identity) live together in the
    'natural_log_exp_and_others' set, but the auto-placer matches Exp to
    'exp_and_others' and Ln to 'natural_log', reloading tables (~1.3us) around
    every Ln.  Blank the redundant sets' contents (positions/ids unchanged, so
    the runtime mapping into act_info.json stays valid) so every activation
    resolves to the one shared set and the table loads once."""
    from concourse.hw_specs import get_activation_tables
    tabs = get_activation_tables(nc.m.arch)
    keep = "natural_log_exp_and_others"
    if keep in tabs:
        for name_, funcs in tabs.items():
            if name_ != keep:
                funcs.clear()


def build():
    nc = bacc.Bacc(None, target_bir_lowering=False, num_devices=8)
    _pin_act_tables(nc)

    # host supplies x pre-transposed [C, N] bf16; weights pre-arranged
    x_ext = nc.declare_dram_parameter("x", [C, N], BF16, isOutput=False)
    wq_ext = nc.declare_dram_parameter("wq", [128, NCC, DQ], BF16, isOutput=False)
    wk_ext = nc.declare_dram_parameter("wk", [128, NCC, DKV], BF16, isOutput=False)
    wv_ext = nc.declare_dram_parameter("wv", [128, NCC, DKV], BF16, isOutput=False)
    wo_ext = nc.declare_dram_parameter("wo", [128, 2, 4, DQ], BF16, isOutput=False)
    out_ext = nc.declare_dram_parameter("out", [N, DQ], BF16, isOutput=True)
    if DEBUG:
        dbg_q = nc.declare_dram_parameter("dbg_q", [128, NQT, N], BF16, isOutput=True)
        dbg_kA = nc.declare_dram_parameter("dbg_kA", [128, N], BF16, isOutput=True)
        dbg_kB = nc.declare_dram_parameter("dbg_kB", [128, N], BF16, isOutput=True)
        dbg_v = nc.declare_dram_parameter("dbg_v", [128, NTC, 130], BF16, isOutput=True)
        dbg_o = nc.declare_dram_parameter("dbg_o", [128, 4, N], BF16, isOutput=True)
        dbg_r = nc.declare_dram_parameter("dbg_r", [16, 2, 512], F32, isOutput=True)

    tabs = _host_tables()
    cosT_d = nc.inline_tensor(tabs["cosT"], name="cosT_d")
    sinTs_d = nc.inline_tensor(tabs["sinTs"], name="sinTs_d")
    trimask_d = nc.inline_tensor(tabs["trimask"], name="trimask_d")
    ident_d = nc.inline_tensor(tabs["ident"], name="ident_d")
    ones4_d = nc.inline_tensor(tabs["ones4"], name="ones4_d")
    expL_d = nc.inline_tensor(tabs["expL"], name="expL_d")

    with tile.TileContext(nc) as tc:
        with (
            tc.tile_pool(name="dram", bufs=1, space="DRAM") as dram,
            tc.tile_pool(name="persist", bufs=1) as ps,
        ):
            # ---- persistent SBUF tensors ----
            xT = ps.tile([128, NCC, N], BF16, name="xT")
            wq_sb = ps.tile([128, NCC, DQ], BF16, name="wq_sb")
            wk_sb = ps.tile([128, NCC, DKV], BF16, name="wk_sb")
            wv_sb = ps.tile([128, NCC, DKV], BF16, name="wv_sb")
            wo_sb = ps.tile([128, 2, 4, DQ], BF16, name="wo_sb")
            cosT = ps.tile([128, N], BF16, name="cosT")
            sinTs = ps.tile([128, N], BF16, name="sinTs")
            trimask = ps.tile([128, 128], BF16, name="trimask")
            ident = ps.tile([128, 128], BF16, name="ident")
            ones4 = ps.tile([128, NQT, 8], BF16, name="ones4")
            expL = ps.tile([8, NQT, 128], BF16, name="expL")
            eps_sb = ps.tile([128, 1], F32, name="eps_sb")
            kT_raw = ps.tile([128, N], BF16, name="kT_raw")
            kTdA = ps.tile([128, N], BF16, name="kTdA")
            kTdB = ps.tile([128, N], BF16, name="kTdB")
            vT = ps.tile([128, N], BF16, name="vT")
            v_sb = ps.tile([128, NTC, 130], BF16, name="v_sb")   # [V_A|1|V_B|1]
            qT = ps.tile([128, NQT, N], BF16, name="qT")
            oT = ps.tile([128, 4, N], BF16, name="oT")           # normalized O^T

            # ---- stage inputs ----
            for cc in range(NCC):
                nc.sync.dma_start(
                    out=xT[:, cc, :], in_=x_ext[cc * 128:(cc + 1) * 128, :]
                )
            nc.gpsimd.dma_start(out=wk_sb[:], in_=wk_ext[:])
            nc.gpsimd.dma_start(out=wv_sb[:], in_=wv_ext[:])
            nc.gpsimd.dma_start(out=wq_sb[:], in_=wq_ext[:])
            nc.gpsimd.dma_start(out=cosT[:], in_=cosT_d[:])
            nc.gpsimd.dma_start(out=sinTs[:], in_=sinTs_d[:])
            nc.gpsimd.dma_start(out=trimask[:], in_=trimask_d[:])
            nc.gpsimd.dma_start(out=ident[:], in_=ident_d[:])
            nc.gpsimd.dma_start(out=ones4[:], in_=ones4_d[:])
            nc.gpsimd.dma_start(out=expL[:], in_=expL_d[:])
            nc.gpsimd.dma_start(out=wo_sb[:], in_=wo_ext[:])
            nc.gpsimd.memset(eps_sb[:], EPS)
            nc.gpsimd.memset(v_sb[:, :, 64:65], 1.0)
            nc.gpsimd.memset(v_sb[:, :, 129:130], 1.0)

            rs_ins = [
                dram.tile([2, 4, 128, DQ], BF16, name=f"rs_in{j}") for j in range(NQT)
            ]
            rs_outs = [
                dram.tile([4, 128, DQ], BF16, name=f"rs_out{j}") for j in range(NQT)
            ]

            with (
                tc.tile_pool(name="u_psum", bufs=1, space="PSUM") as up,
                tc.tile_pool(name="u_sbuf", bufs=3) as bs,
            ):
                # ---------- helpers ----------
                def emit_proj(w_sb, nf_off, nf, dst, tag_base):
                    """project xT against w_sb[:, :, nf_off:nf_off+nf] and
                    evict to dst [128, N] bf16."""
                    for qp in range(2):
                        pp = up.tile(
                            [128, 2, 512], F32,
                            tag=("mm" if qp % 2 == 0 else "o"), bufs=2,
                            name=f"pp{tag_base}",
                        )
                        for h in range(2):
                            qt = 2 * qp + h
                            for cc in range(NCC):
                                nc.tensor.matmul(
                                    pp[:, h, :],
                                    w_sb[:, cc, nf_off:nf_off + nf],
                                    xT[:, cc, qt * 512:(qt + 1) * 512],
                                    start=(cc == 0), stop=(cc == NCC - 1),
                                )
                        # Act is idle during the projection phase; keep DVE free
                        nc.scalar.copy(
                            dst[:, qp * 1024:(qp + 1) * 1024],
                            pp.rearrange("p a b -> p (a b)"),
                        )

                def emit_rms_rope(src, dst, ci):
                    """rms-norm (no weight) + rope, feature-major, in place ok."""
                    sq = bs.tile([128, N], BF16, tag="sq", bufs=2, name=f"sq{ci}")
                    nc.vector.tensor_mul(sq[:], src, src)
                    msp = up.tile([8, 512], F32, tag="mm", bufs=2, name=f"msp{ci}")
                    for qt in range(NQT):
                        nc.tensor.matmul(
                            msp[:], ones4[:, qt, :], sq[:, qt * 512:(qt + 1) * 512],
                            start=(qt == 0), stop=(qt == NQT - 1),
                        )
                    lnv = bs.tile([8, 512], F32, tag="lnv", bufs=2, name=f"lnv{ci}")
                    nc.scalar.activation(
                        lnv[:], msp[:], AF.Ln, bias=eps_sb[0:8, :], scale=1.0 / D
                    )
                    rr2 = bs.tile([8, 512], BF16, tag="rr2", bufs=2, name=f"rr2{ci}")
                    nc.scalar.activation(rr2[:], lnv[:], AF.Exp, scale=-0.5)
                    t1 = bs.tile([128, N], BF16, tag="t1", bufs=2, name=f"t1{ci}")
                    nc.vector.tensor_mul(t1[:], src, cosT[:])
                    t2 = bs.tile([128, N], BF16, tag="t2", bufs=2, name=f"t2{ci}")
                    nc.vector.tensor_copy(t2[0:32, :], src[32:64, :])
                    nc.vector.tensor_copy(t2[32:64, :], src[0:32, :])
                    nc.vector.tensor_copy(t2[64:96, :], src[96:128, :])
                    nc.vector.tensor_copy(t2[96:128, :], src[64:96, :])
                    nc.vector.tensor_mul(t2[:], t2[:], sinTs[:])
                    nc.vector.tensor_add(t1[:], t1[:], t2[:])
                    for qt in range(NQT):
                        rbp = up.tile(
                            [128, 512], F32, tag="o", bufs=2, name=f"rbp{ci}"
                        )
                        nc.tensor.matmul(
                            rbp[:], expL[:, qt, :], rr2[:],
                            start=True, stop=True,
                        )
                        nc.vector.tensor_mul(
                            dst[:, qt * 512:(qt + 1) * 512],
                            t1[:, qt * 512:(qt + 1) * 512], rbp[:],
                        )

                def emit_attention(j, m):
                    kT = kTdA if m < 2 else kTdB
                    vslot = 0 if m < 2 else 65
                    oab = up.tile([65, 2, 512], F32, tag="o", bufs=2, name=f"oab{j}_{m}")
                    nkc = 4 * (j + 1)

                    def emit_scores(kc):
                        i = kc - 4 * j
                        off = max(0, 128 * i)
                        w = 512 - off
                        q0 = 512 * j + off
                        sAB = up.tile([128, 2, 512], F32, tag="mm", bufs=2, name=f"sAB{kc}")
                        nc.tensor.matmul(
                            sAB[:, 0, 0:w], kT[0:64, kc * 128:(kc + 1) * 128],
                            qT[0:64, m, q0:q0 + w], start=True, stop=True,
                            tile_position=(0, 0),
                        )
                        nc.tensor.matmul(
                            sAB[:, 1, 0:w], kT[64:128, kc * 128:(kc + 1) * 128],
                            qT[64:128, m, q0:q0 + w], start=True, stop=True,
                            tile_position=(64, 0),
                        )
                        pAB = bs.tile([128, 2, 512], BF16, tag="pAB", bufs=5, name=f"pAB{kc}")
                        nc.scalar.activation(
                            pAB[:, :, 0:w], sAB[:, :, 0:w], AF.Exp, scale=EXP_SCALE
                        )
                        if i >= 0:
                            nc.vector.tensor_mul(
                                pAB[:, :, 0:128], pAB[:, :, 0:128],
                                trimask.rearrange("p (a b) -> p a b", a=1).broadcast_to([128, 2, 128]),
                            )
                        return pAB

                    def emit_pv(kc, pAB):
                        i = kc - 4 * j
                        off = max(0, 128 * i)
                        w = 512 - off
                        nc.tensor.matmul(
                            oab[:, 0, off:512], v_sb[:, kc, vslot:vslot + 65],
                            pAB[:, 0, 0:w], start=(kc == 0), stop=(kc == nkc - 1),
                            skip_group_check=True,
                        )
                        nc.tensor.matmul(
                            oab[:, 1, off:512], v_sb[:, kc, vslot:vslot + 65],
                            pAB[:, 1, 0:w], start=(kc == 0), stop=(kc == nkc - 1),
                            skip_group_check=True,
                        )

                    staged = []
                    for kc in range(nkc):
                        staged.append((kc, emit_scores(kc)))
                        if len(staged) == 2:
                            for kcx, px in staged:
                                emit_pv(kcx, px)
                            staged = []
                    for kcx, px in staged:
                        emit_pv(kcx, px)
                    # normalize: r = 1/rowsum broadcast over 64 partitions
                    ssum = bs.tile([1, 2, 512], F32, tag="ssum", bufs=2, name=f"ssum{j}_{m}")
                    nc.vector.tensor_copy(ssum[:], oab[64:65, :, :])
                    rrf = bs.tile([1, 2, 512], F32, tag="rrf", bufs=2, name=f"rrf{j}_{m}")
                    nc.vector.reciprocal_approx_fast(rrf[:], ssum[:])
                    if DEBUG:
                        nc.sync.dma_start(out=dbg_r[4 * j + m], in_=rrf[0, :, :])
                    rbs = bs.tile([64, 1024], F32, tag="rbs", bufs=2, name=f"rbs{j}_{m}")
                    nc.gpsimd.partition_broadcast(
                        rbs[:], rrf.rearrange("p a b -> p (a b)"), channels=64
                    )
                    nc.vector.tensor_mul(
                        oT[0:64, m, 512 * j:512 * (j + 1)], oab[0:64, 0, :], rbs[:, 0:512]
                    )
                    nc.vector.tensor_mul(
                        oT[64:128, m, 512 * j:512 * (j + 1)], oab[0:64, 1, :], rbs[:, 512:1024]
                    )

                def emit_outproj(j, tt):
                    """partial y for both column-halves from own heads; the
                    pairwise ReduceScatter-add fires once per j-tile (4 token
                    chunks batched) so only 4 collectives hit the CC engine."""
                    tcix = j * 4 + tt
                    po = up.tile([128, 2, 512], F32, tag="o", bufs=2, name=f"po{tcix}")
                    for s in range(2):
                        for rc in range(4):
                            nc.tensor.matmul(
                                po[:, s, :],
                                oT[:, rc, tcix * 128:(tcix + 1) * 128],
                                wo_sb[:, s, rc, :],
                                start=(rc == 0), stop=(rc == 3),
                            )
                    ev = bs.tile([128, 2, 512], BF16, tag="ev", bufs=2, name=f"ev{tcix}")
                    nc.vector.tensor_copy(ev[:], po[:])
                    nc.sync.dma_start(
                        out=rs_ins[j].rearrange("s t p f -> p s t f")[:, :, tt, :],
                        in_=ev[:],
                    )
                    if tt == 3:
                        nc.gpsimd.collective_compute(
                            "ReduceScatter",
                            mybir.AluOpType.add,
                            replica_groups=PAIRS,
                            ins=[rs_ins[j].opt()],
                            outs=[rs_outs[j].opt()],
                        )
                        nc.sync.dma_start(
                            out=out_ext[j * 512:(j + 1) * 512, :],
                            in_=rs_outs[j].rearrange("t p f -> (t p) f"),
                        )

                # ---------- emission ----------
                # K/V projections + k rope/rms + V transpose
                emit_proj(wk_sb, 0, DKV, kT_raw, "k")
                emit_proj(wv_sb, 0, DKV, vT, "v")
                for tcix in range(NTC):
                    pv = up.tile(
                        [128, 128], BF16,
                        tag=("mm" if tcix % 2 == 0 else "o"), bufs=2, name="pv",
                    )
                    nc.tensor.transpose(pv[:], vT[:, tcix * 128:(tcix + 1) * 128], ident[:])
                    nc.vector.tensor_copy(v_sb[:, tcix, 0:64], pv[:, 0:64])
                    nc.vector.tensor_copy(v_sb[:, tcix, 65:129], pv[:, 64:128])
                emit_rms_rope(kT_raw[:], kTdA[:], 4)
                # duplicate kv halves: kTdA holds [A; B] -> kTdA=[A;A], kTdB=[B;B]
                nc.vector.tensor_copy(kTdB[0:64, :], kTdA[64:128, :])
                nc.vector.tensor_copy(kTdB[64:128, :], kTdA[64:128, :])
                nc.vector.tensor_copy(kTdA[64:128, :], kTdA[0:64, :])

                # Q per m-group, interleaved with first attention tile
                for m in range(4):
                    emit_proj(wq_sb, m * 128, 128, qT[:, m, :], f"q{m}")
                    emit_rms_rope(qT[:, m, :], qT[:, m, :], m)
                    if m > 0:
                        emit_attention(0, m - 1)
                emit_attention(0, 3)

                for j in range(1, NQT):
                    for m in range(4):
                        emit_attention(j, m)
                        emit_outproj(j - 1, m)
                for tt in range(4):
                    emit_outproj(NQT - 1, tt)
                if DEBUG:
                    nc.sync.dma_start(out=dbg_q[:], in_=qT[:])
                    nc.sync.dma_start(out=dbg_kA[:], in_=kTdA[:])
                    nc.sync.dma_start(out=dbg_kB[:], in_=kTdB[:])
                    nc.sync.dma_start(out=dbg_v[:], in_=v_sb[:])
                    nc.sync.dma_start(out=dbg_o[:], in_=oT[:])

    nc.finalize()
    return nc


_NC_CACHE = None


def _get_nc():
    global _NC_CACHE
    if _NC_CACHE is None:
        _NC_CACHE = build()
    return _NC_CACHE


def _make_in_maps(inputs):
    x = np.asarray(inputs["x"], dtype=np.float32)
    wq = np.asarray(inputs["wq"], dtype=np.float32)
    wk = np.asarray(inputs["wk"], dtype=np.float32)
    wv = np.asarray(inputs["wv"], dtype=np.float32)
    wo = np.asarray(inputs["wo"], dtype=np.float32)
    bf = ml_dtypes.bfloat16

    def warr(w, p, nf):
        """[C, nf] local slice -> [128, NCC, nf] bf16"""
        wl = w[:, p * nf:(p + 1) * nf]
        return np.ascontiguousarray(
            wl.reshape(NCC, 128, nf).transpose(1, 0, 2).astype(bf)
        )

    in_maps = []
    for i in range(8):
        b, p = i // 2, i % 2
        wo_l = np.empty((128, 2, 4, DQ), dtype=bf)
        for s in range(2):
            for rc in range(4):
                wo_l[:, s, rc, :] = wo[
                    p * DQ + rc * 128:p * DQ + (rc + 1) * 128,
                    s * DQ:(s + 1) * DQ,
                ].astype(bf)
        in_maps.append({
            "x": np.ascontiguousarray(x[b].T.astype(bf)),
            "wq": warr(wq, p, DQ),
            "wk": warr(wk, p, DKV),
            "wv": warr(wv, p, DKV),
            "wo": np.ascontiguousarray(wo_l),
        })
    return in_maps


def kernel(x, wq, wk, wv, wo):
    x = np.asarray(x, dtype=np.float32)
    B = x.shape[0]
    nc = _get_nc()
    in_maps = _make_in_maps({"x": x, "wq": wq, "wk": wk, "wv": wv, "wo": wo})
    res = run_bass_kernel_spmd(nc, in_maps, core_ids=list(range(8)))
    out = np.empty((B, N, C), dtype=np.float32)
    for b in range(B):
        out[b, :, 0:DQ] = res.results[2 * b]["out"].astype(np.float32)
        out[b, :, DQ:C] = res.results[2 * b + 1]["out"].astype(np.float32)
    return out


if __name__ == "__main__":
    rng = np.random.default_rng(0)
    ins = {
        "x": rng.standard_normal((4, N, C), dtype=np.float32),
        "wq": (rng.standard_normal((C, C), dtype=np.float32) * 0.02),
        "wk": (rng.standard_normal((C, 256), dtype=np.float32) * 0.02),
        "wv": (rng.standard_normal((C, 256), dtype=np.float32) * 0.02),
        "wo": (rng.standard_normal((C, C), dtype=np.float32) * 0.02),
    }
    y = kernel(**ins)
    print("out", y.shape, y.dtype, np.abs(y).mean())
